# revision 10
# baseline (speedup 1.0000x reference)
"""GQA FlashAttention (RMSNorm QK + RoPE, causal) on 8 TRN2 NeuronCores.

Sharding: tensor-parallel over heads for QKV projection + attention
(core c owns q-heads 4c..4c+3 and kv-head c — the GQA group is fully
local, so attention needs no collective). A single AllToAll re-shards
the attention output from head-parallel to seq-row-parallel, after
which each core computes its 256 output rows against the full Wo
(no all-reduce). Softmax uses the unnormalized-exp trick: denominators
come free from a ones-column appended to V, and the division is applied
to the small attention output after the PV matmul.

All matmuls run in float32r (fp32 storage, ~4x fp32 PE rate; measured
same precision as the fp32 PE path). Everything is computed in the
transposed layout (head_dim on partitions) so the scores output IS the
P^T operand the PV matmul needs — zero transposes in the attention
inner loop.
"""

import sys

sys.path.insert(0, "/opt/trn_rl_repo")

import numpy as np
import concourse.bass as bass  # noqa: F401  (engine types referenced via nc)
import concourse.tile as tile
from concourse import mybir, bacc
from concourse.bass_utils import run_bass_kernel_spmd
from concourse.masks import make_identity

N_CORES = 8
D_IN = 2048
SEQ = 2048
N_HEADS = 32
N_KV = 8
HD = 64
HPC = N_HEADS // N_CORES  # 4 q heads per core
EPS = 1e-6
NEG = -1.0e9

F32 = mybir.dt.float32
F32R = mybir.dt.float32r

KT_TILES = D_IN // 128  # 16 contraction tiles for projections
QB = 512  # q block (matmul moving dim)
NQB = SEQ // QB  # 4
NKT = SEQ // 128  # 16 kv tiles
ROWS_PER_CORE = SEQ // N_CORES  # 256


def _build():
    nc = bacc.Bacc(num_devices=N_CORES)

    xT = nc.dram_tensor("xT", [D_IN, SEQ], F32R, kind="ExternalInput")
    wq = nc.dram_tensor("wq", [D_IN, HPC * HD], F32R, kind="ExternalInput")
    wkv = nc.dram_tensor("wkv", [D_IN, 2 * HD], F32R, kind="ExternalInput")
    wo = nc.dram_tensor("wo", [D_IN, D_IN], F32R, kind="ExternalInput")
    cosT2 = nc.dram_tensor("cosT2", [128, SEQ], F32, kind="ExternalInput")
    sinT2 = nc.dram_tensor("sinT2", [128, SEQ], F32, kind="ExternalInput")
    qw2 = nc.dram_tensor("qw2", [128, 1], F32, kind="ExternalInput")
    kw = nc.dram_tensor("kw", [64, 1], F32, kind="ExternalInput")
    tri = nc.dram_tensor("tri", [128, 128], F32, kind="ExternalInput")
    sel = nc.dram_tensor("sel", [4 * N_CORES, 2 * N_CORES, 128], F32R, kind="ExternalInput")
    onesblk_in = nc.dram_tensor("onesblk", [128, 128], F32R, kind="ExternalInput")
    onescol_in = nc.dram_tensor("onescol", [128, 1], F32R, kind="ExternalInput")

    out = nc.dram_tensor("out", [ROWS_PER_CORE, D_IN], F32, kind="ExternalOutput")

    with tile.TileContext(nc) as tc:
        with (
            tc.tile_pool(name="persist", bufs=1) as pers,
            tc.tile_pool(name="dram", bufs=1, space="DRAM") as dram,
        ):
            # ---- persistent SBUF ----
            wq_sb = pers.tile([128, KT_TILES, HPC * HD], F32R)  # 2 MB
            nc.sync.dma_start(
                wq_sb[:], wq.rearrange("(ko p) m -> p ko m", p=128)
            )
            wkv_sb = pers.tile([128, KT_TILES, 2 * HD], F32R)  # 1 MB
            nc.sync.dma_start(
                wkv_sb[:], wkv.rearrange("(ko p) m -> p ko m", p=128)
            )
            cos_sb = pers.tile([128, SEQ], F32)
            sin_sb = pers.tile([128, SEQ], F32)
            nc.sync.dma_start(cos_sb[:], cosT2[:])
            nc.sync.dma_start(sin_sb[:], sinT2[:])
            qw_sb = pers.tile([128, 1], F32)
            kw_sb = pers.tile([64, 1], F32)
            nc.sync.dma_start(qw_sb[:], qw2[:])
            nc.sync.dma_start(kw_sb[:], kw[:])
            tri_sb = pers.tile([128, 128], F32)
            nc.sync.dma_start(tri_sb[:], tri[:])
            eps_sb = pers.tile([128, 1], F32)
            nc.vector.memset(eps_sb[:], EPS)
            sel_sb = pers.tile([4 * N_CORES, 2 * N_CORES, 128], F32R)
            nc.sync.dma_start(sel_sb[:], sel[:])

            ident = pers.tile([128, 128], F32)
            make_identity(nc, ident[:])

            # block-diagonal ones (two 64x64 blocks) for per-head sumsq+broadcast
            onesblk = pers.tile([128, 128], F32R)
            nc.sync.dma_start(onesblk[:], onesblk_in[:])

            # QT per head at base partition 0: [64, 4 heads, SEQ]
            qt_sb = pers.tile([64, HPC, SEQ], F32R)  # 2 MB
            kt_sb = pers.tile([64, SEQ], F32R)  # 0.5 MB
            vaug_sb = pers.tile([128, NKT, HD + 1], F32R)  # 0.53 MB
            for _t in range(NKT):
                nc.sync.dma_start(vaug_sb[:, _t, HD : HD + 1], onescol_in[:])

            # DRAM scratch for the AllToAll
            a2a_in = dram.tile([N_CORES, HPC * HD + HPC, ROWS_PER_CORE], F32)
            a2a_out = dram.tile([N_CORES, HPC * HD + HPC, ROWS_PER_CORE], F32)

            # ================= Phase 1: projections + norm + rope =============
            with (
                tc.tile_pool(name="xt", bufs=4) as xp,
                tc.tile_pool(name="p1ps", bufs=2, space="PSUM") as psA,
                tc.tile_pool(name="p1sb", bufs=3) as t1,
            ):
                for j in range(NQB):
                    sl = slice(QB * j, QB * j + QB)
                    acc = [
                        psA.tile([128, QB], F32, tag="acc0", name=f"acc0_{j}"),
                        psA.tile([128, QB], F32, tag="acc1", name=f"acc1_{j}"),
                        psA.tile([128, QB], F32, tag="acc2", name=f"acc2_{j}"),
                    ]
                    for k in range(KT_TILES):
                        xt = xp.tile([128, QB], F32R, tag="xt")
                        nc.sync.dma_start(
                            xt[:], xT[128 * k : 128 * k + 128, sl]
                        )
                        st = k == 0
                        sp = k == KT_TILES - 1
                        nc.tensor.matmul(
                            acc[0][:], wq_sb[:, k, 0:128], xt[:], start=st, stop=sp
                        )
                        nc.tensor.matmul(
                            acc[1][:], wq_sb[:, k, 128:256], xt[:], start=st, stop=sp
                        )
                        nc.tensor.matmul(
                            acc[2][:], wkv_sb[:, k, :], xt[:], start=st, stop=sp
                        )

                    for idx in range(3):
                        raw = acc[idx]
                        is_kv = idx == 2
                        rows = slice(0, 64) if is_kv else slice(0, 128)
                        # sumsq broadcast per head (block-diag ones matmul)
                        sq = t1.tile([128, QB], F32R, tag="sq")
                        nc.scalar.square(sq[:], raw[:])
                        psn = psA.tile([128, QB], F32, tag="norm", bufs=1)
                        nc.tensor.matmul(
                            psn[:], onesblk[:], sq[:], start=True, stop=True
                        )
                        rcp = t1.tile([128, QB], F32, tag="rcp")
                        nc.scalar.activation(
                            out=rcp[rows, :],
                            in_=psn[rows, :],
                            func=mybir.ActivationFunctionType.Sqrt,
                            bias=eps_sb[rows, :],
                            scale=1.0 / HD,
                        )
                        nc.vector.reciprocal(rcp[rows, :], rcp[rows, :])
                        # normalized = raw * rcp * norm_w
                        tn = t1.tile([128, QB], F32, tag="tn")
                        nc.vector.tensor_mul(tn[rows, :], raw[rows, :], rcp[rows, :])
                        if is_kv:
                            nc.vector.tensor_scalar_mul(
                                tn[0:64, :], tn[0:64, :], kw_sb[:]
                            )
                        else:
                            nc.vector.tensor_scalar_mul(tn[:], tn[:], qw_sb[:])
                        # rope: rot = [-t[32:64], t[0:32]] per 64-row head
                        rot = t1.tile([128, QB], F32, tag="rot")
                        nheads_here = 1 if is_kv else 2
                        for b in range(nheads_here):
                            o = 64 * b
                            nc.vector.tensor_scalar_mul(
                                rot[o : o + 32, :], tn[o + 32 : o + 64, :], -1.0
                            )
                            nc.vector.tensor_copy(
                                rot[o + 32 : o + 64, :], tn[o : o + 32, :]
                            )
                        if is_kv:
                            dst = kt_sb[:, sl]
                            nc.vector.tensor_mul(dst, tn[0:64, :], cos_sb[0:64, sl])
                            nc.vector.tensor_mul(
                                rot[0:64, :], rot[0:64, :], sin_sb[0:64, sl]
                            )
                            nc.vector.tensor_add(dst, dst, rot[0:64, :])
                            # V rows: evict + transpose to natural layout
                            vt = t1.tile([64, QB], F32, tag="vt")
                            nc.scalar.copy(vt[:], raw[64:128, :])
                            for ttl in range(QB // 128):
                                tg = (QB // 128) * j + ttl
                                psv = psA.tile([128, 64], F32, tag="vtr", bufs=1)
                                nc.tensor.transpose(
                                    psv[:],
                                    vt[:, 128 * ttl : 128 * ttl + 128],
                                    ident[0:64, 0:64],
                                )
                                nc.scalar.copy(vaug_sb[:, tg, 0:HD], psv[:])
                        else:
                            tmpc = t1.tile([128, QB], F32, tag="tmpc")
                            nc.vector.tensor_mul(tmpc[:], tn[:], cos_sb[:, sl])
                            nc.vector.tensor_mul(rot[:], rot[:], sin_sb[:, sl])
                            for b in range(2):
                                nc.vector.tensor_add(
                                    qt_sb[:, 2 * idx + b, sl],
                                    tmpc[64 * b : 64 * b + 64, :],
                                    rot[64 * b : 64 * b + 64, :],
                                )

            # ================= Phase 3: attention =============================
            with (
                tc.tile_pool(name="p3ps", bufs=3, space="PSUM") as psB,
                tc.tile_pool(name="p3pv", bufs=2, space="PSUM") as psPV,
                tc.tile_pool(name="p3sb", bufs=3) as t3,
            ):
                for h in range(HPC):
                    for j in range(NQB):
                        ntile = (QB // 128) * (j + 1)
                        pv = psPV.tile([128, QB], F32, tag="pv")
                        for t in range(ntile):
                            diag_m = t - (QB // 128) * j
                            ks = slice(128 * t, 128 * t + 128)
                            if diag_m < 0:
                                qs = slice(QB * j, QB * j + QB)
                                n0 = 0
                            else:
                                n0 = 128 * diag_m
                                qs = slice(QB * j + n0, QB * j + QB)
                            ps_s = psB.tile([128, QB], F32, tag="sc")
                            nc.tensor.matmul(
                                ps_s[:, 0 : QB - n0],
                                kt_sb[:, ks],
                                qt_sb[:, h, qs],
                                start=True,
                                stop=True,
                            )
                            if diag_m >= 0:
                                nc.vector.tensor_add(
                                    ps_s[:, 0:128], ps_s[:, 0:128], tri_sb[:]
                                )
                            pt = t3.tile([128, QB], F32R, tag="pt")
                            nc.scalar.activation(
                                out=pt[:, 0 : QB - n0],
                                in_=ps_s[:, 0 : QB - n0],
                                func=mybir.ActivationFunctionType.Exp,
                                scale=0.125,
                            )
                            nc.tensor.matmul(
                                pv[0:65, n0:QB],
                                vaug_sb[:, t, :],
                                pt[:, 0 : QB - n0],
                                start=(t == 0),
                                stop=(t == ntile - 1),
                            )
                        att = t3.tile([65, QB], F32, tag="att")
                        nc.scalar.copy(att[:], pv[0:65, :])
                        for s in range(QB // ROWS_PER_CORE):
                            shard = (QB // ROWS_PER_CORE) * j + s
                            cs = slice(ROWS_PER_CORE * s, ROWS_PER_CORE * (s + 1))
                            nc.sync.dma_start(
                                a2a_in[shard, 64 * h : 64 * h + 64, :],
                                att[0:64, cs],
                            )
                            nc.sync.dma_start(
                                a2a_in[shard, HPC * 64 + h, :], att[64:65, cs]
                            )

            # ================= Phase 4: AllToAll ==============================
            nc.gpsimd.collective_compute(
                "AllToAll",
                mybir.AluOpType.bypass,
                replica_groups=[list(range(N_CORES))],
                ins=[a2a_in[:].opt()],
                outs=[a2a_out[:].opt()],
            )

            # ================= Phase 5: out projection ========================
            with (
                tc.tile_pool(name="p5ps", bufs=2, space="PSUM") as psC,
                tc.tile_pool(name="p5bc", bufs=2, space="PSUM") as psD,
                tc.tile_pool(name="wo", bufs=8) as wop,
                tc.tile_pool(name="p5sb", bufs=4) as t5,
                tc.tile_pool(name="an", bufs=1) as anp,
            ):
                R = ROWS_PER_CORE
                dsb_raw = t5.tile([4 * N_CORES, R], F32, tag="denraw")
                for g in range(N_CORES):
                    nc.sync.dma_start(
                        dsb_raw[4 * g : 4 * g + 4, :],
                        a2a_out[g, HPC * 64 : HPC * 64 + 4, :],
                    )
                nc.vector.reciprocal(dsb_raw[:], dsb_raw[:])
                dsb = t5.tile([4 * N_CORES, R], F32R, tag="den")
                nc.vector.tensor_copy(dsb[:], dsb_raw[:])

                an_sb = anp.tile([128, 2 * N_CORES, R], F32R)  # normalized attnT
                for g in range(N_CORES):
                    for half in range(2):
                        a_raw = t5.tile([128, R], F32, tag="araw")
                        nc.sync.dma_start(
                            a_raw[:], a2a_out[g, 128 * half : 128 * half + 128, :]
                        )
                        bc = psD.tile([128, R], F32, tag="bc")
                        nc.tensor.matmul(
                            bc[:],
                            sel_sb[:, 2 * g + half, :],
                            dsb[:],
                            start=True,
                            stop=True,
                        )
                        nc.vector.tensor_mul(
                            an_sb[:, 2 * g + half, :], a_raw[:], bc[:]
                        )

                NB_OUT = D_IN // 512  # 4
                for nb in range(NB_OUT):
                    osl = slice(512 * nb, 512 * nb + 512)
                    po = [
                        psC.tile([128, 512], F32, tag="o0", name=f"o0_{nb}"),
                        psC.tile([128, 512], F32, tag="o1", name=f"o1_{nb}"),
                    ]
                    for gh in range(2 * N_CORES):
                        wt = wop.tile([128, 512], F32R, tag="wo")
                        nc.sync.dma_start(
                            wt[:], wo[128 * gh : 128 * gh + 128, osl]
                        )
                        for qt in range(2):
                            nc.tensor.matmul(
                                po[qt][:],
                                an_sb[:, gh, 128 * qt : 128 * qt + 128],
                                wt[:],
                                start=(gh == 0),
                                stop=(gh == 2 * N_CORES - 1),
                            )
                    for qt in range(2):
                        osb = t5.tile([128, 512], F32, tag="osb")
                        nc.scalar.copy(osb[:], po[qt][:])
                        nc.sync.dma_start(
                            out[128 * qt : 128 * qt + 128, osl], osb[:]
                        )

    nc.compile()
    return nc


_NC_CACHE = None


def _get_nc():
    global _NC_CACHE
    if _NC_CACHE is None:
        _NC_CACHE = _build()
    return _NC_CACHE


def _make_in_maps(x, cos, sin, wq, wk, wv, wo, q_norm_w, k_norm_w):
    x = np.asarray(x, dtype=np.float32)
    cos = np.asarray(cos, dtype=np.float32)
    sin = np.asarray(sin, dtype=np.float32)
    wq = np.asarray(wq, dtype=np.float32)
    wk = np.asarray(wk, dtype=np.float32)
    wv = np.asarray(wv, dtype=np.float32)
    wo = np.asarray(wo, dtype=np.float32)
    q_norm_w = np.asarray(q_norm_w, dtype=np.float32)
    k_norm_w = np.asarray(k_norm_w, dtype=np.float32)

    xT = np.ascontiguousarray(x[0].T)  # [D_IN, SEQ]
    cosT2 = np.ascontiguousarray(np.vstack([cos.T, cos.T]))  # [128, SEQ]
    sinT2 = np.ascontiguousarray(np.vstack([sin.T, sin.T]))
    qw2 = np.ascontiguousarray(np.concatenate([q_norm_w, q_norm_w])[:, None])
    kw1 = np.ascontiguousarray(k_norm_w[:, None])
    ii, jj = np.meshgrid(np.arange(128), np.arange(128), indexing="ij")
    tri = np.where(ii <= jj, 0.0, NEG).astype(np.float32)  # keep kv<=q
    onesblk = np.zeros((128, 128), np.float32)
    onesblk[0:64, 0:64] = 1.0
    onesblk[64:128, 64:128] = 1.0
    onescol = np.ones((128, 1), np.float32)
    sel = np.zeros((4 * N_CORES, 2 * N_CORES, 128), np.float32)
    for g in range(N_CORES):
        for half in range(2):
            for m in range(128):
                sel[4 * g + 2 * half + m // 64, 2 * g + half, m] = 1.0

    in_maps = []
    for c in range(N_CORES):
        wq_c = np.ascontiguousarray(wq[:, 256 * c : 256 * c + 256])
        wkv_c = np.ascontiguousarray(
            np.concatenate(
                [wk[:, 64 * c : 64 * c + 64], wv[:, 64 * c : 64 * c + 64]], axis=1
            )
        )
        in_maps.append(
            {
                "xT": xT,
                "wq": wq_c,
                "wkv": wkv_c,
                "wo": wo,
                "cosT2": cosT2,
                "sinT2": sinT2,
                "qw2": qw2,
                "kw": kw1,
                "tri": tri,
                "sel": sel,
                "onesblk": onesblk,
                "onescol": onescol,
            }
        )
    return in_maps


def kernel(x, cos, sin, wq, wk, wv, wo, q_norm_w, k_norm_w):
    in_maps = _make_in_maps(x, cos, sin, wq, wk, wv, wo, q_norm_w, k_norm_w)
    nc = _get_nc()
    res = run_bass_kernel_spmd(nc, in_maps, core_ids=list(range(N_CORES)))
    rows = [res.results[c]["out"] for c in range(N_CORES)]
    full = np.concatenate(rows, axis=0)  # [SEQ, D_IN]
    return full.reshape(1, SEQ, D_IN).astype(np.float32)
